# revision 32
# baseline (speedup 1.0000x reference)
"""Trainium2 Bass kernel for nn_Block_7584912244953 (gnn_message_passing).

Strategy (8 NeuronCores, SPMD, node-row sharding, no collectives):
  - Associativity: Wp1 @ (e @ W) == (Wp1 @ e) @ W. Each core computes
    T_b^T = e^T @ Wp1_b[rows]^T for its 512 node rows (contraction over
    all 16384 edges), then small (512x512) projections.
  - RMS norms folded into streamed operands host-side; gains and the
    softmax 1/sqrt(D) folded into weights.
  - Branch order hh -> ee+eh -> he so each branch's per-node 8x8
    attention (DVE) hides under the next branch's big PE contraction;
    the tail (sdpa_he -> RMS -> transpose -> FFN) is pipelined per
    128-row tile so PE keeps working while DVE drains.
  - Stream DMAs batched 4 k-tiles per issue (SP issue rate is ~0.6us).
  - SDPA segmented reductions as fp16 halving trees (DVE 2x mode) +
    one short fp32 reduce; TensorReduce has no fast mode on TRN2.
  - fp16 operands throughout the PE paths (same speed as bf16, 8x finer
    mantissa); exp() stays bf16 for range; accumulation fp32 in PSUM.
"""

import numpy as np
import ml_dtypes

NCORES = 8
H, D = 8, 64
_CACHE = {}


def _dims(scale=1):
    N, M, E = 4096 // scale, 16384 // scale, 512
    R = N // NCORES
    return dict(N=N, M=M, E=E, R=R, NT=R // 128, ET=E // 128, MT=M // 128,
                NMT=N // 128, F=4 * E, FT=4 * E // 128)


def _build(scale=1, loopn=1, sim_safe=False):
    import concourse.bacc as bacc
    import concourse.mybir as mybir
    from concourse import tile

    dm = _dims(scale)
    N, M, E, R = dm["N"], dm["M"], dm["E"], dm["R"]
    NT, ET, MT, NMT, F, FT = dm["NT"], dm["ET"], dm["MT"], dm["NMT"], dm["F"], dm["FT"]
    MB, NB = MT // 4, NMT // 4

    F32 = mybir.dt.float32
    I32 = mybir.dt.int32
    F16 = mybir.dt.float16
    B16 = mybir.dt.bfloat16
    AF = mybir.ActivationFunctionType
    ALU = mybir.AluOpType
    AX = mybir.AxisListType

    nc = bacc.Bacc("TRN2", target_bir_lowering=False, debug=False, num_devices=NCORES)

    d_xe = nc.dram_tensor("xe4", [MB * 128, 4 * E], F16, kind="ExternalInput")
    d_wp = {b: nc.dram_tensor(f"wp4_{b}", [MB * 128, 4 * R], F16, kind="ExternalInput")
            for b in ("ee", "eh", "he")}
    d_adj = nc.dram_tensor("adj4", [NB * 128, 4 * R], F16, kind="ExternalInput")
    d_xn = nc.dram_tensor("xn4", [NB * 128, 4 * E], F16, kind="ExternalInput")
    d_xnt = nc.dram_tensor("xnt4", [128, ET * R], F16, kind="ExternalInput")
    WT_B = ["q_hh", "q_ee", "k_ee", "q_eh", "k_he"]
    WN_B = ["k_hh", "v_hh", "v_ee", "k_eh", "v_eh", "q_he", "v_he"]
    d_w = {w: nc.dram_tensor(f"w_{w}", [128, ET * E], F16, kind="ExternalInput")
           for w in WT_B + WN_B}
    d_wf1 = nc.dram_tensor("wf1", [FT * 128, ET * 128], F16, kind="ExternalInput")
    d_b1t = nc.dram_tensor("b1t", [128, FT], F32, kind="ExternalInput")
    d_wf2 = nc.dram_tensor("wf2", [(F + 128) // 128 * 128, E], F16, kind="ExternalInput")
    d_id = nc.dram_tensor("ident", [128, 128], F16, kind="ExternalInput")
    d_ones = nc.dram_tensor("onesrow", [1, 128], F16, kind="ExternalInput")
    d_out = nc.dram_tensor("out", [R, E], F32, kind="ExternalOutput")

    with tile.TileContext(nc) as tc:
        with (
            tc.tile_pool(name="stream", bufs=2) as st,
            tc.tile_pool(name="wst", bufs=2) as ws,
            tc.tile_pool(name="qkv", bufs=2) as qs,
            tc.tile_pool(name="tstore", bufs=1) as ts_,
            tc.tile_pool(name="xnts", bufs=1) as xs,
            tc.tile_pool(name="sdpa", bufs=1) as sp,
            tc.tile_pool(name="sdpa2", bufs=2) as sp2,
            tc.tile_pool(name="xacc", bufs=1) as xa,
            tc.tile_pool(name="psum", bufs=1, space="PSUM") as pp,
            tc.tile_pool(name="misc", bufs=1) as mp,
            tc.tile_pool(name="ffn", bufs=3) as fs,
        ):
            def body(iv=None):
                qkv = {}
                tstore = {}

                # ================= T_hh pass (adj contraction), banks 4-7 =====
                ps_hh = [pp.tile([128, R], F32, tag=f"bank{4 + e}", name=f"pshh{e}")
                         for e in range(ET)]
                for b in range(NB):
                    xn_t = st.tile([128, 4 * E], F16, tag="s_xe")
                    nc.sync.dma_start(out=xn_t[:], in_=d_xn.ap()[b * 128:(b + 1) * 128, :])
                    ad_t = st.tile([128, 4 * R], F16, tag="s_wa")
                    nc.sync.dma_start(out=ad_t[:], in_=d_adj.ap()[b * 128:(b + 1) * 128, :])
                    for i in range(4):
                        for e in range(ET):
                            nc.tensor.matmul(ps_hh[e][:], xn_t[:, i * E + e * 128: i * E + (e + 1) * 128],
                                             ad_t[:, i * R:(i + 1) * R],
                                             start=(b == 0 and i == 0), stop=(b == NB - 1 and i == 3))

                # ---- residents (issued after the first pass's stream DMAs)
                xnt = xs.tile([128, ET * R], F16, tag="xnt")
                nc.sync.dma_start(out=xnt[:], in_=d_xnt.ap())
                identb = mp.tile([128, 128], F16, tag="identb")
                nc.sync.dma_start(out=identb[:], in_=d_id.ap())
                eps_t = mp.tile([128, 1], F32, tag="eps")
                nc.gpsimd.memset(eps_t[:], 1e-6)
                x_tiles = [xa.tile([128, E], F32, tag=f"x{t}", name=f"x{t}") for t in range(NT)]

                for e in range(ET):
                    tt = ts_.tile([128, R], F16, tag=f"Thh{e}")
                    nc.scalar.copy(tt[:], ps_hh[e][:])
                    tstore[("hh", e)] = tt

                def load_w(name):
                    w = ws.tile([128, ET * E], F16, tag="w_s")
                    nc.sync.dma_start(out=w[:], in_=d_w[name].ap())
                    return w

                def src_T(branch):
                    return lambda k, t: tstore[(branch, k)][:, t * 128:(t + 1) * 128]

                def src_xnt(k, t):
                    return xnt[:, k * R + t * 128: k * R + t * 128 + 128]

                def proj_group(specs, banks, tiles=None):
                    """specs: (out_name, srcf(k,t)->lhsT AP, w_tile [128, ET*E])."""
                    for t in (range(NT) if tiles is None else tiles):
                        psb = [pp.tile([128, E], F32, tag=f"bank{banks[i]}",
                                       name=f"ps_{name}")
                               for i, (name, _, _) in enumerate(specs)]
                        for k in range(ET):
                            for i, (name, srcf, w) in enumerate(specs):
                                nc.tensor.matmul(psb[i][:], srcf(k, t),
                                                 w[:, k * E:(k + 1) * E],
                                                 start=(k == 0), stop=(k == ET - 1))
                        for i, (name, srcf, w) in enumerate(specs):
                            q = qs.tile([128, E], F16, tag=f"{name[0]}{t}")
                            nc.scalar.copy(q[:], psb[i][:])
                            qkv[(name, t)] = q

                # ---- SDPA on DVE/ACT
                def sdpa(branch, t, first):
                    qb = qkv[(f"q_{branch}", t)]
                    kb = qkv[(f"k_{branch}", t)]
                    vb = qkv[(f"v_{branch}", t)]
                    P = sp.tile([128, H * H * D], F16, tag="P")
                    q_ap = qb[:].rearrange("p (h d) -> p h d", h=H).unsqueeze(2).broadcast_to((128, H, H, D))
                    k_ap = kb[:].rearrange("p (g d) -> p g d", g=H).unsqueeze(1).broadcast_to((128, H, H, D))
                    nc.vector.tensor_tensor(out=P[:].rearrange("p (h g d) -> p h g d", h=H, g=H),
                                            in0=q_ap, in1=k_ap, op=ALU.mult)
                    # s[h,g] = sum_d P: three fp16 halving levels + fp32 reduce over 8
                    cur, width = P, D
                    for lvl in range(3):
                        nxt = sp2.tile([128, H * H * width // 2], F16, tag=f"str{lvl}")
                        v_ = cur[:].rearrange("p (s d) -> p s d", d=width)
                        nc.vector.tensor_tensor(
                            out=nxt[:].rearrange("p (s d) -> p s d", d=width // 2),
                            in0=v_[:, :, 0:width // 2], in1=v_[:, :, width // 2:width],
                            op=ALU.add)
                        cur, width = nxt, width // 2
                    s_f = sp2.tile([128, H * H], F32, tag="s")
                    nc.vector.reduce_sum(out=s_f[:],
                                         in_=cur[:].rearrange("p (s d) -> p s d", d=width),
                                         axis=AX.X)
                    Eb = sp2.tile([128, H * H], B16, tag="Eb")
                    nc.scalar.activation(Eb[:], s_f[:], AF.Exp)
                    den = sp2.tile([128, H], F32, tag="den")
                    nc.vector.reduce_sum(out=den[:], in_=Eb[:].rearrange("p (h g) -> p h g", g=H),
                                         axis=AX.X)
                    rec = sp2.tile([128, H], F32, tag="rec")
                    nc.vector.reciprocal(rec[:], den[:])
                    EbN = sp2.tile([128, H * H], F16, tag="EbN")
                    nc.vector.tensor_tensor(out=EbN[:].rearrange("p (h g) -> p h g", h=H),
                                            in0=Eb[:].rearrange("p (h g) -> p h g", h=H),
                                            in1=rec[:].unsqueeze(2).broadcast_to((128, H, H)),
                                            op=ALU.mult)
                    Pa = sp.tile([128, H * D * H], F16, tag="Pa")
                    e_ap = EbN[:].rearrange("p (h g) -> p h g", h=H).unsqueeze(2).broadcast_to((128, H, D, H))
                    v_ap = vb[:].rearrange("p (d g) -> p d g", g=H).unsqueeze(1).broadcast_to((128, H, D, H))
                    nc.vector.tensor_tensor(out=Pa[:].rearrange("p (h d g) -> p h d g", h=H, d=D),
                                            in0=e_ap, in1=v_ap, op=ALU.mult)
                    # x[h,d] (+)= sum_g Pa: two fp16 halving levels + final pair-add
                    cur, width = Pa, H
                    for lvl in range(2):
                        nxt = sp2.tile([128, H * D * width // 2], F16, tag=f"atr{lvl}")
                        v_ = cur[:].rearrange("p (s g) -> p s g", g=width)
                        nc.vector.tensor_tensor(
                            out=nxt[:].rearrange("p (s g) -> p s g", g=width // 2),
                            in0=v_[:, :, 0:width // 2], in1=v_[:, :, width // 2:width],
                            op=ALU.add)
                        cur, width = nxt, width // 2
                    v_ = cur[:].rearrange("p (s g) -> p s g", g=2)
                    a0 = v_[:, :, 0:1].rearrange("p s o -> p (s o)")
                    a1 = v_[:, :, 1:2].rearrange("p s o -> p (s o)")
                    xt = x_tiles[t]
                    if first:
                        nc.vector.tensor_tensor(out=xt[:], in0=a0, in1=a1, op=ALU.add)
                    else:
                        tmp = sp2.tile([128, E], F32, tag="tmp")
                        nc.vector.tensor_tensor(out=tmp[:], in0=a0, in1=a1, op=ALU.add)
                        nc.vector.tensor_tensor(out=xt[:], in0=xt[:], in1=tmp[:], op=ALU.add)

                # ---- branch hh projections + SDPA (overlaps pass1 below)
                w_qhh = load_w("q_hh")
                w_khh = load_w("k_hh")
                w_vhh = load_w("v_hh")
                proj_group([("q_hh", src_T("hh"), w_qhh),
                            ("k_hh", src_xnt, w_khh),
                            ("v_hh", src_xnt, w_vhh)], banks=[0, 1, 2])
                for t in range(NT):
                    sdpa("hh", t, first=True)

                # ================= pass1: T_ee + T_eh, banks 0-3 / 4-7 ========
                ps_ee = [pp.tile([128, R], F32, tag=f"bank{e}", name=f"psee{e}")
                         for e in range(ET)]
                ps_eh = [pp.tile([128, R], F32, tag=f"bank{4 + e}", name=f"pseh{e}")
                         for e in range(ET)]
                for b in range(MB):
                    xe_t = st.tile([128, 4 * E], F16, tag="s_xe")
                    nc.sync.dma_start(out=xe_t[:], in_=d_xe.ap()[b * 128:(b + 1) * 128, :])
                    wee = st.tile([128, 4 * R], F16, tag="s_wa")
                    nc.sync.dma_start(out=wee[:], in_=d_wp["ee"].ap()[b * 128:(b + 1) * 128, :])
                    weh = st.tile([128, 4 * R], F16, tag="s_wb")
                    nc.sync.dma_start(out=weh[:], in_=d_wp["eh"].ap()[b * 128:(b + 1) * 128, :])
                    for i in range(4):
                        for e in range(ET):
                            lt = xe_t[:, i * E + e * 128: i * E + (e + 1) * 128]
                            st_ = (b == 0 and i == 0)
                            sp_ = (b == MB - 1 and i == 3)
                            nc.tensor.matmul(ps_ee[e][:], lt, wee[:, i * R:(i + 1) * R],
                                             start=st_, stop=sp_)
                            nc.tensor.matmul(ps_eh[e][:], lt, weh[:, i * R:(i + 1) * R],
                                             start=st_, stop=sp_)
                for e in range(ET):
                    tt = ts_.tile([128, R], F16, tag=f"Tee{e}")
                    nc.scalar.copy(tt[:], ps_ee[e][:])
                    tstore[("ee", e)] = tt
                    tt2 = ts_.tile([128, R], F16, tag=f"Teh{e}")
                    nc.scalar.copy(tt2[:], ps_eh[e][:])
                    tstore[("eh", e)] = tt2

                # ---- branches ee + eh projections + SDPA (overlap pass2)
                w_qee = load_w("q_ee")
                w_kee = load_w("k_ee")
                w_qeh = load_w("q_eh")
                proj_group([("q_ee", src_T("ee"), w_qee),
                            ("k_ee", src_T("ee"), w_kee),
                            ("q_eh", src_T("eh"), w_qeh)], banks=[0, 1, 2])
                w_vee = load_w("v_ee")
                w_keh = load_w("k_eh")
                w_veh = load_w("v_eh")
                proj_group([("v_ee", src_xnt, w_vee),
                            ("k_eh", src_xnt, w_keh),
                            ("v_eh", src_xnt, w_veh)], banks=[0, 1, 2])
                for t in range(NT):
                    sdpa("ee", t, first=False)
                for t in range(NT):
                    sdpa("eh", t, first=False)

                # ================= pass2: T_he, banks 4-7 =====================
                # Split into two node-column halves so the first half's
                # projections + sdpa_he(t0,t1) overlap the second half's PE.
                the_tiles = [ts_.tile([128, R], F16, tag=f"The{e}", name=f"The{e}")
                             for e in range(ET)]
                for e in range(ET):
                    tstore[("he", e)] = the_tiles[e]

                def pass2_part(c0, Rw):
                    """T_he for node cols [c0, c0+Rw); PSUM evacuated on DVE
                    (the ACT queue head-blocks behind pending sdpa exps)."""
                    cs = slice(c0, c0 + Rw)
                    ps_he = [pp.tile([128, Rw], F32, tag=f"bank{4 + e}", name=f"pshe{e}")
                             for e in range(ET)]
                    for b in range(MB):
                        xe_t = st.tile([128, 4 * E], F16, tag="s_xe")
                        nc.sync.dma_start(out=xe_t[:], in_=d_xe.ap()[b * 128:(b + 1) * 128, :])
                        whe = st.tile([128, 4 * Rw], F16, tag="s_wa")
                        nc.sync.dma_start(
                            out=whe[:],
                            in_=d_wp["he"].ap()[b * 128:(b + 1) * 128, :]
                            .rearrange("p (i r) -> p i r", i=4)[:, :, cs])
                        for i in range(4):
                            for e in range(ET):
                                nc.tensor.matmul(ps_he[e][:],
                                                 xe_t[:, i * E + e * 128: i * E + (e + 1) * 128],
                                                 whe[:, i * Rw:(i + 1) * Rw],
                                                 start=(b == 0 and i == 0), stop=(b == MB - 1 and i == 3))
                    for e in range(ET):
                        nc.scalar.copy(the_tiles[e][:, cs], ps_he[e][:])

                # ---- branch he T-side projection; tail pipelined per tile
                w_khe = load_w("k_he")

                def ffn_residents():
                    """Issued only after pass2's stream DMAs: the 16KB slab
                    loads occupy the SP queue ~6us each and pass2 is already
                    near the HBM bandwidth limit."""
                    wf1r = xs.tile([128, FT * ET * 128], F16, tag="wf1r")
                    nc.sync.dma_start(
                        out=wf1r[:].rearrange("p (f c) -> p f c", f=FT),
                        in_=d_wf1.ap().rearrange("(f p) c -> p f c", f=FT))
                    wf2r = xs.tile([128, FT * E], F16, tag="wf2r")
                    nc.sync.dma_start(
                        out=wf2r[:].rearrange("p (f c) -> p f c", f=FT),
                        in_=d_wf2.ap()[0:F, :].rearrange("(f p) c -> p f c", f=FT))
                    b1 = mp.tile([128, FT], F32, tag="b1")
                    nc.sync.dma_start(out=b1[:], in_=d_b1t.ap())
                    ones_t = mp.tile([1, 128], F16, tag="ones")
                    nc.sync.dma_start(out=ones_t[:], in_=d_ones.ap())
                    wtb = mp.tile([128, E], F16, tag="wf2b")
                    nc.sync.dma_start(out=wtb[:], in_=d_wf2.ap()[F:F + 128, :])
                    return wf1r, wf2r, b1, ones_t, wtb

                yT = [xa.tile([128, R], F16, tag=f"yT{e}", name=f"yT{e}") for e in range(ET)]
                y_tiles = {}

                def rms(t):
                    xt = x_tiles[t]
                    scr = sp2.tile([128, E], F32, tag="tmp")
                    nc.scalar.activation(scr[:], xt[:], AF.Square)
                    ms = sp2.tile([128, 1], F32, tag="ms")
                    nc.vector.reduce_sum(out=ms[:], in_=scr[:], axis=AX.X)
                    sd = sp2.tile([128, 1], F32, tag="sd")
                    nc.scalar.activation(sd[:], ms[:], AF.Sqrt, scale=1.0 / E, bias=eps_t[:])
                    inv2 = sp2.tile([128, 1], F32, tag="inv")
                    nc.vector.reciprocal(inv2[:], sd[:])
                    yt = sp.tile([128, E], F16, tag="y")
                    nc.vector.tensor_scalar_mul(yt[:], xt[:], inv2[:])
                    y_tiles[t] = yt

                def transpose_tile(t):
                    yt = y_tiles[t]
                    for e in range(ET):
                        pst = pp.tile([128, 128], F16, tag=f"bank{2 + (e % 2)}")
                        nc.tensor.transpose(pst[:], yt[:, e * 128:(e + 1) * 128], identb[:])
                        nc.scalar.copy(yT[e][:, t * 128:(t + 1) * 128], pst[:])

                def ffn_tile(t):
                    wf1r, wf2r, b1, ones_t, wtb = ffn_res
                    pso = pp.tile([128, E], F32, tag=f"bank{4 + t}", name=f"pso{t}")
                    for f in range(FT):
                        psz = pp.tile([128, 128], F32, tag=f"bank{f % 2}")
                        for k in range(ET):
                            nc.tensor.matmul(psz[:],
                                             wf1r[:, f * E + k * 128: f * E + (k + 1) * 128],
                                             yT[k][:, t * 128:(t + 1) * 128],
                                             start=(k == 0), stop=(k == ET - 1))
                        zt = fs.tile([128, 128], F16, tag="zT")
                        nc.scalar.activation(zt[:], psz[:],
                                             AF.Identity if sim_safe else AF.Gelu,
                                             bias=b1[:, f:f + 1])
                        nc.tensor.matmul(pso[:], zt[:], wf2r[:, f * E:(f + 1) * E],
                                         start=(f == 0), stop=False)
                    nc.tensor.matmul(pso[:], ones_t[0:1, :], wtb[0:1, :], start=False, stop=True)
                    ot = sp.tile([128, E], F32, tag="ot")
                    nc.scalar.copy(ot[:], pso[:])
                    nc.sync.dma_start(out=d_out.ap()[t * 128:(t + 1) * 128, :], in_=ot[:])

                pass2_part(0, R)
                ffn_res = ffn_residents()
                w_qhe = load_w("q_he")
                w_vhe = load_w("v_he")
                proj_group([("k_he", src_T("he"), w_khe),
                            ("q_he", src_xnt, w_qhe),
                            ("v_he", src_xnt, w_vhe)], banks=[0, 1, 2])
                for t in range(NT):
                    sdpa("he", t, first=False)
                    rms(t)
                    transpose_tile(t)
                    ffn_tile(t)

            if loopn > 1:
                with tc.For_i(0, loopn, 1) as _i:
                    body(_i)
            else:
                body()

    nc.compile()
    return nc


def _prep_inputs(inputs, scale=1):
    """Host-side folding + sharding. Returns per-core in_maps."""
    dm = _dims(scale)
    N, M, E, R, F, FT, ET = dm["N"], dm["M"], dm["E"], dm["R"], dm["F"], dm["FT"], dm["ET"]
    x_node = np.asarray(inputs["x_node"], np.float32)
    x_edge = np.asarray(inputs["x_edge"], np.float32)
    adj = np.asarray(inputs["adj"], np.float32)
    g_n = np.asarray(inputs["g_n"], np.float32)
    g_e = np.asarray(inputs["g_e"], np.float32)
    g2 = np.asarray(inputs["g2"], np.float32)

    inv_n = (1.0 / np.sqrt((x_node.astype(np.float64) ** 2).mean(axis=1) + 1e-6)).astype(np.float32)
    inv_e = (1.0 / np.sqrt((x_edge.astype(np.float64) ** 2).mean(axis=1) + 1e-6)).astype(np.float32)
    xn_s = x_node * inv_n[:, None]
    xe_s = x_edge * inv_e[:, None]

    perm = np.array([(j % H) * D + j // H for j in range(E)])  # newcol j=(d,g) <- oldcol g*D+d

    def fold_q(w, g):
        return (g[:, None] * np.asarray(w, np.float32)) / np.sqrt(D)

    def fold_k(w, g):
        return g[:, None] * np.asarray(w, np.float32)

    def fold_v(w, g):
        return (g[:, None] * np.asarray(w, np.float32))[:, perm]

    wmap = {
        "q_hh": fold_q(inputs["Wq_hh"], g_n),
        "q_ee": fold_q(inputs["Wq_ee"], g_e),
        "k_ee": fold_k(inputs["Wk_ee"], g_e),
        "q_eh": fold_q(inputs["Wq_eh"], g_e),
        "k_he": fold_k(inputs["Wk_he"], g_e),
        "k_hh": fold_k(inputs["Wk_hh"], g_n),
        "v_hh": fold_v(inputs["Wv_hh"], g_n),
        "v_ee": fold_v(inputs["Wv_ee"], g_n),
        "k_eh": fold_k(inputs["Wk_eh"], g_n),
        "v_eh": fold_v(inputs["Wv_eh"], g_n),
        "q_he": fold_q(inputs["Wq_he"], g_n),
        "v_he": fold_v(inputs["Wv_he"], g_n),
    }
    wf1 = g2[:, None] * np.asarray(inputs["Wf1"], np.float32)
    bf1 = np.asarray(inputs["bf1"], np.float32)
    b1t = np.ascontiguousarray(bf1.reshape(FT, 128).T)
    wf2e = np.zeros((F + 128, E), np.float32)
    wf2e[:F] = np.asarray(inputs["Wf2"], np.float32)
    wf2e[F] = np.asarray(inputs["bf2"], np.float32)

    def batch4(a, width):
        """[T*128, width] -> [T/4*128, 4*width]: 4 consecutive k-tiles side by side."""
        T = a.shape[0] // 128
        return np.ascontiguousarray(
            a.reshape(T // 4, 4, 128, width).transpose(0, 2, 1, 3).reshape(T // 4 * 128, 4 * width))

    def warr(w):
        """[E, E] -> [128, ET*E]: k-tiles along columns."""
        return np.ascontiguousarray(
            np.asarray(w).reshape(ET, 128, E).transpose(1, 0, 2).reshape(128, ET * E))

    shared = {
        "xe4": batch4(xe_s.astype(np.float16), E),
        "xn4": batch4(xn_s.astype(np.float16), E),
        "b1t": b1t,
        "wf1": np.ascontiguousarray(
            wf1.reshape(ET, 128, FT, 128).transpose(2, 1, 0, 3).reshape(FT * 128, ET * 128)).astype(np.float16),
        "wf2": wf2e.astype(np.float16),
        "ident": np.eye(128, dtype=np.float16),
        "onesrow": np.ones((1, 128), np.float16),
    }
    for k, v in wmap.items():
        shared[f"w_{k}"] = warr(v).astype(np.float16)

    wp1 = {b: np.asarray(inputs[f"Wp1_{b}"], np.float32) for b in ("ee", "eh", "he")}
    in_maps = []
    for c in range(NCORES):
        rows = slice(c * R, (c + 1) * R)
        m = dict(shared)
        m["adj4"] = batch4(np.ascontiguousarray(adj[rows].T).astype(np.float16), R)
        xnt = np.ascontiguousarray(xn_s[rows].T)  # [E, R]
        m["xnt4"] = np.ascontiguousarray(
            xnt.reshape(ET, 128, R).transpose(1, 0, 2).reshape(128, ET * R)).astype(np.float16)
        for b in ("ee", "eh", "he"):
            m[f"wp4_{b}"] = batch4(np.ascontiguousarray(wp1[b][rows].T).astype(np.float16), R)
        in_maps.append(m)
    return in_maps


def kernel(**inputs) -> np.ndarray:
    from concourse.bass_utils import run_bass_kernel_spmd

    if "nc" not in _CACHE:
        _CACHE["nc"] = _build()
    nc = _CACHE["nc"]
    in_maps = _prep_inputs(inputs)
    res = run_bass_kernel_spmd(nc, in_maps, list(range(NCORES)))
    out = np.concatenate([res.results[c]["out"] for c in range(NCORES)], axis=0)
    return np.ascontiguousarray(out, dtype=np.float32)


# revision 34
# speedup vs baseline: 1.0288x; 1.0288x over previous
"""Trainium2 Bass kernel for nn_Block_7584912244953 (gnn_message_passing).

Strategy (8 NeuronCores, SPMD, node-row sharding, no collectives):
  - Associativity: Wp1 @ (e @ W) == (Wp1 @ e) @ W. Each core computes
    T_b^T = e^T @ Wp1_b[rows]^T for its 512 node rows (contraction over
    all 16384 edges), then small (512x512) projections.
  - RMS norms folded into streamed operands host-side; gains and the
    softmax 1/sqrt(D) folded into weights.
  - Branch order hh -> ee+eh -> he so each branch's per-node 8x8
    attention (DVE) hides under the next branch's big PE contraction;
    the tail (sdpa_he -> RMS -> transpose -> FFN) is pipelined per
    128-row tile so PE keeps working while DVE drains.
  - Stream DMAs batched 4 k-tiles per issue (SP issue rate is ~0.6us).
  - SDPA segmented reductions as fp16 halving trees (DVE 2x mode) +
    one short fp32 reduce; TensorReduce has no fast mode on TRN2.
  - fp16 operands throughout the PE paths (same speed as bf16, 8x finer
    mantissa); exp() stays bf16 for range; accumulation fp32 in PSUM.
"""

import numpy as np
import ml_dtypes

NCORES = 8
H, D = 8, 64
_CACHE = {}


def _dims(scale=1):
    N, M, E = 4096 // scale, 16384 // scale, 512
    R = N // NCORES
    return dict(N=N, M=M, E=E, R=R, NT=R // 128, ET=E // 128, MT=M // 128,
                NMT=N // 128, F=4 * E, FT=4 * E // 128)


def _build(scale=1, loopn=1, sim_safe=False):
    import concourse.bacc as bacc
    import concourse.mybir as mybir
    from concourse import tile

    dm = _dims(scale)
    N, M, E, R = dm["N"], dm["M"], dm["E"], dm["R"]
    NT, ET, MT, NMT, F, FT = dm["NT"], dm["ET"], dm["MT"], dm["NMT"], dm["F"], dm["FT"]
    MB, NB = MT // 4, NMT // 4

    F32 = mybir.dt.float32
    I32 = mybir.dt.int32
    F16 = mybir.dt.float16
    B16 = mybir.dt.bfloat16
    AF = mybir.ActivationFunctionType
    ALU = mybir.AluOpType
    AX = mybir.AxisListType

    nc = bacc.Bacc("TRN2", target_bir_lowering=False, debug=False, num_devices=NCORES)

    d_xe = nc.dram_tensor("xe4", [MB * 128, 4 * E], F16, kind="ExternalInput")
    d_wp = {b: nc.dram_tensor(f"wp4_{b}", [MB * 128, 4 * R], F16, kind="ExternalInput")
            for b in ("ee", "eh", "he")}
    d_adj = nc.dram_tensor("adj4", [NB * 128, 4 * R], F16, kind="ExternalInput")
    d_xn = nc.dram_tensor("xn4", [NB * 128, 4 * E], F16, kind="ExternalInput")
    d_xnt = nc.dram_tensor("xnt4", [128, ET * R], F16, kind="ExternalInput")
    WT_B = ["q_hh", "q_ee", "k_ee", "q_eh", "k_he"]
    WN_B = ["k_hh", "v_hh", "v_ee", "k_eh", "v_eh", "q_he", "v_he"]
    d_w = {w: nc.dram_tensor(f"w_{w}", [128, ET * E], F16, kind="ExternalInput")
           for w in WT_B + WN_B}
    d_wf1 = nc.dram_tensor("wf1", [FT * 128, ET * 128], F16, kind="ExternalInput")
    d_b1t = nc.dram_tensor("b1t", [128, FT], F32, kind="ExternalInput")
    d_wf2 = nc.dram_tensor("wf2", [(F + 128) // 128 * 128, E], F16, kind="ExternalInput")
    d_id = nc.dram_tensor("ident", [128, 128], F16, kind="ExternalInput")
    d_ones = nc.dram_tensor("onesrow", [1, 128], F16, kind="ExternalInput")
    d_out = nc.dram_tensor("out", [R, E], F32, kind="ExternalOutput")

    with tile.TileContext(nc) as tc:
        with (
            tc.tile_pool(name="stream", bufs=2) as st,
            tc.tile_pool(name="wst", bufs=2) as ws,
            tc.tile_pool(name="qkv", bufs=2) as qs,
            tc.tile_pool(name="tstore", bufs=1) as ts_,
            tc.tile_pool(name="xnts", bufs=1) as xs,
            tc.tile_pool(name="sdpa", bufs=1) as sp,
            tc.tile_pool(name="sdpa2", bufs=2) as sp2,
            tc.tile_pool(name="xacc", bufs=1) as xa,
            tc.tile_pool(name="psum", bufs=1, space="PSUM") as pp,
            tc.tile_pool(name="misc", bufs=1) as mp,
            tc.tile_pool(name="ffn", bufs=3) as fs,
        ):
            def body(iv=None):
                qkv = {}
                tstore = {}

                # ================= T_hh pass (adj contraction), banks 4-7 =====
                ps_hh = [pp.tile([128, R], F32, tag=f"bank{4 + e}", name=f"pshh{e}")
                         for e in range(ET)]
                for b in range(NB):
                    xn_t = st.tile([128, 4 * E], F16, tag="s_xe")
                    nc.sync.dma_start(out=xn_t[:], in_=d_xn.ap()[b * 128:(b + 1) * 128, :])
                    ad_t = st.tile([128, 4 * R], F16, tag="s_wa")
                    nc.sync.dma_start(out=ad_t[:], in_=d_adj.ap()[b * 128:(b + 1) * 128, :])
                    for i in range(4):
                        for e in range(ET):
                            nc.tensor.matmul(ps_hh[e][:], xn_t[:, i * E + e * 128: i * E + (e + 1) * 128],
                                             ad_t[:, i * R:(i + 1) * R],
                                             start=(b == 0 and i == 0), stop=(b == NB - 1 and i == 3))

                # ---- residents (issued after the first pass's stream DMAs)
                xnt = xs.tile([128, ET * R], F16, tag="xnt")
                nc.sync.dma_start(out=xnt[:], in_=d_xnt.ap())
                identb = mp.tile([128, 128], F16, tag="identb")
                nc.sync.dma_start(out=identb[:], in_=d_id.ap())
                eps_t = mp.tile([128, 1], F32, tag="eps")
                nc.gpsimd.memset(eps_t[:], 1e-6)
                x_tiles = [xa.tile([128, E], F32, tag=f"x{t}", name=f"x{t}") for t in range(NT)]

                for e in range(ET):
                    tt = ts_.tile([128, R], F16, tag=f"Thh{e}")
                    nc.scalar.copy(tt[:], ps_hh[e][:])
                    tstore[("hh", e)] = tt

                def load_w(name):
                    w = ws.tile([128, ET * E], F16, tag="w_s")
                    nc.sync.dma_start(out=w[:], in_=d_w[name].ap())
                    return w

                def src_T(branch):
                    return lambda k, t: tstore[(branch, k)][:, t * 128:(t + 1) * 128]

                def src_xnt(k, t):
                    return xnt[:, k * R + t * 128: k * R + t * 128 + 128]

                def proj_group(specs, banks, tiles=None):
                    """specs: (out_name, srcf(k,t)->lhsT AP, w_tile [128, ET*E])."""
                    for t in (range(NT) if tiles is None else tiles):
                        psb = [pp.tile([128, E], F32, tag=f"bank{banks[i]}",
                                       name=f"ps_{name}")
                               for i, (name, _, _) in enumerate(specs)]
                        for k in range(ET):
                            for i, (name, srcf, w) in enumerate(specs):
                                nc.tensor.matmul(psb[i][:], srcf(k, t),
                                                 w[:, k * E:(k + 1) * E],
                                                 start=(k == 0), stop=(k == ET - 1))
                        for i, (name, srcf, w) in enumerate(specs):
                            q = qs.tile([128, E], F16, tag=f"{name[0]}{t}")
                            nc.scalar.copy(q[:], psb[i][:])
                            qkv[(name, t)] = q

                # ---- SDPA on DVE/ACT
                def sdpa(branch, t, first):
                    qb = qkv[(f"q_{branch}", t)]
                    kb = qkv[(f"k_{branch}", t)]
                    vb = qkv[(f"v_{branch}", t)]
                    P = sp.tile([128, H * H * D], F16, tag="P")
                    q_ap = qb[:].rearrange("p (h d) -> p h d", h=H).unsqueeze(2).broadcast_to((128, H, H, D))
                    k_ap = kb[:].rearrange("p (g d) -> p g d", g=H).unsqueeze(1).broadcast_to((128, H, H, D))
                    nc.vector.tensor_tensor(out=P[:].rearrange("p (h g d) -> p h g d", h=H, g=H),
                                            in0=q_ap, in1=k_ap, op=ALU.mult)
                    # s[h,g] = sum_d P: three fp16 halving levels + fp32 reduce over 8
                    cur, width = P, D
                    for lvl in range(3):
                        nxt = sp2.tile([128, H * H * width // 2], F16, tag=f"str{lvl}")
                        v_ = cur[:].rearrange("p (s d) -> p s d", d=width)
                        nc.vector.tensor_tensor(
                            out=nxt[:].rearrange("p (s d) -> p s d", d=width // 2),
                            in0=v_[:, :, 0:width // 2], in1=v_[:, :, width // 2:width],
                            op=ALU.add)
                        cur, width = nxt, width // 2
                    s_f = sp2.tile([128, H * H], F32, tag="s")
                    nc.vector.reduce_sum(out=s_f[:],
                                         in_=cur[:].rearrange("p (s d) -> p s d", d=width),
                                         axis=AX.X)
                    Eb = sp2.tile([128, H * H], B16, tag="Eb")
                    nc.scalar.activation(Eb[:], s_f[:], AF.Exp)
                    den = sp2.tile([128, H], F32, tag="den")
                    nc.vector.reduce_sum(out=den[:], in_=Eb[:].rearrange("p (h g) -> p h g", g=H),
                                         axis=AX.X)
                    rec = sp2.tile([128, H], F32, tag="rec")
                    nc.vector.reciprocal(rec[:], den[:])
                    EbN = sp2.tile([128, H * H], F16, tag="EbN")
                    nc.vector.tensor_tensor(out=EbN[:].rearrange("p (h g) -> p h g", h=H),
                                            in0=Eb[:].rearrange("p (h g) -> p h g", h=H),
                                            in1=rec[:].unsqueeze(2).broadcast_to((128, H, H)),
                                            op=ALU.mult)
                    Pa = sp.tile([128, H * D * H], F16, tag="Pa")
                    e_ap = EbN[:].rearrange("p (h g) -> p h g", h=H).unsqueeze(2).broadcast_to((128, H, D, H))
                    v_ap = vb[:].rearrange("p (d g) -> p d g", g=H).unsqueeze(1).broadcast_to((128, H, D, H))
                    nc.vector.tensor_tensor(out=Pa[:].rearrange("p (h d g) -> p h d g", h=H, d=D),
                                            in0=e_ap, in1=v_ap, op=ALU.mult)
                    # x[h,d] (+)= sum_g Pa: two fp16 halving levels + final pair-add
                    cur, width = Pa, H
                    for lvl in range(2):
                        nxt = sp2.tile([128, H * D * width // 2], F16, tag=f"atr{lvl}")
                        v_ = cur[:].rearrange("p (s g) -> p s g", g=width)
                        nc.vector.tensor_tensor(
                            out=nxt[:].rearrange("p (s g) -> p s g", g=width // 2),
                            in0=v_[:, :, 0:width // 2], in1=v_[:, :, width // 2:width],
                            op=ALU.add)
                        cur, width = nxt, width // 2
                    v_ = cur[:].rearrange("p (s g) -> p s g", g=2)
                    a0 = v_[:, :, 0:1].rearrange("p s o -> p (s o)")
                    a1 = v_[:, :, 1:2].rearrange("p s o -> p (s o)")
                    xt = x_tiles[t]
                    if first:
                        nc.vector.tensor_tensor(out=xt[:], in0=a0, in1=a1, op=ALU.add)
                    else:
                        tmp = sp2.tile([128, E], F32, tag="tmp")
                        nc.vector.tensor_tensor(out=tmp[:], in0=a0, in1=a1, op=ALU.add)
                        nc.vector.tensor_tensor(out=xt[:], in0=xt[:], in1=tmp[:], op=ALU.add)

                # ---- branch hh projections + SDPA (overlaps pass1 below)
                w_qhh = load_w("q_hh")
                w_khh = load_w("k_hh")
                w_vhh = load_w("v_hh")
                proj_group([("q_hh", src_T("hh"), w_qhh),
                            ("k_hh", src_xnt, w_khh),
                            ("v_hh", src_xnt, w_vhh)], banks=[0, 1, 2])
                for t in range(NT):
                    sdpa("hh", t, first=True)

                # ================= pass1: T_ee + T_eh, banks 0-3 / 4-7 ========
                ps_ee = [pp.tile([128, R], F32, tag=f"bank{e}", name=f"psee{e}")
                         for e in range(ET)]
                ps_eh = [pp.tile([128, R], F32, tag=f"bank{4 + e}", name=f"pseh{e}")
                         for e in range(ET)]
                # First KEEP xe blocks stay resident in SBUF for pass2's
                # re-read: pass2 otherwise demands ~308GB/s of the ~358GB/s
                # per-core HBM budget and stretches on hardware.
                KEEP = max(MB // 4, 1)
                xe_keep = {}
                for b in range(MB):
                    if b < KEEP:
                        xe_t = xs.tile([128, 4 * E], F16, tag=f"xr{b}", name=f"xr{b}")
                        xe_keep[b] = xe_t
                    else:
                        xe_t = st.tile([128, 4 * E], F16, tag="s_xe")
                    nc.sync.dma_start(out=xe_t[:], in_=d_xe.ap()[b * 128:(b + 1) * 128, :])
                    wee = st.tile([128, 4 * R], F16, tag="s_wa")
                    nc.sync.dma_start(out=wee[:], in_=d_wp["ee"].ap()[b * 128:(b + 1) * 128, :])
                    weh = st.tile([128, 4 * R], F16, tag="s_wb")
                    nc.sync.dma_start(out=weh[:], in_=d_wp["eh"].ap()[b * 128:(b + 1) * 128, :])
                    for i in range(4):
                        for e in range(ET):
                            lt = xe_t[:, i * E + e * 128: i * E + (e + 1) * 128]
                            st_ = (b == 0 and i == 0)
                            sp_ = (b == MB - 1 and i == 3)
                            nc.tensor.matmul(ps_ee[e][:], lt, wee[:, i * R:(i + 1) * R],
                                             start=st_, stop=sp_)
                            nc.tensor.matmul(ps_eh[e][:], lt, weh[:, i * R:(i + 1) * R],
                                             start=st_, stop=sp_)
                for e in range(ET):
                    tt = ts_.tile([128, R], F16, tag=f"Tee{e}")
                    nc.scalar.copy(tt[:], ps_ee[e][:])
                    tstore[("ee", e)] = tt
                    tt2 = ts_.tile([128, R], F16, tag=f"Teh{e}")
                    nc.scalar.copy(tt2[:], ps_eh[e][:])
                    tstore[("eh", e)] = tt2

                # ---- branches ee + eh projections + SDPA (overlap pass2)
                w_qee = load_w("q_ee")
                w_kee = load_w("k_ee")
                w_qeh = load_w("q_eh")
                proj_group([("q_ee", src_T("ee"), w_qee),
                            ("k_ee", src_T("ee"), w_kee),
                            ("q_eh", src_T("eh"), w_qeh)], banks=[0, 1, 2])
                w_vee = load_w("v_ee")
                w_keh = load_w("k_eh")
                w_veh = load_w("v_eh")
                proj_group([("v_ee", src_xnt, w_vee),
                            ("k_eh", src_xnt, w_keh),
                            ("v_eh", src_xnt, w_veh)], banks=[0, 1, 2])
                for t in range(NT):
                    sdpa("ee", t, first=False)
                for t in range(NT):
                    sdpa("eh", t, first=False)

                # ================= pass2: T_he, banks 4-7 =====================
                # Split into two node-column halves so the first half's
                # projections + sdpa_he(t0,t1) overlap the second half's PE.
                the_tiles = [ts_.tile([128, R], F16, tag=f"The{e}", name=f"The{e}")
                             for e in range(ET)]
                for e in range(ET):
                    tstore[("he", e)] = the_tiles[e]

                def pass2_part(c0, Rw):
                    """T_he for node cols [c0, c0+Rw); PSUM evacuated on DVE
                    (the ACT queue head-blocks behind pending sdpa exps)."""
                    cs = slice(c0, c0 + Rw)
                    ps_he = [pp.tile([128, Rw], F32, tag=f"bank{4 + e}", name=f"pshe{e}")
                             for e in range(ET)]
                    for b in range(MB):
                        if b in xe_keep:
                            xe_t = xe_keep[b]
                        else:
                            xe_t = st.tile([128, 4 * E], F16, tag="s_xe")
                            nc.sync.dma_start(out=xe_t[:], in_=d_xe.ap()[b * 128:(b + 1) * 128, :])
                        whe = st.tile([128, 4 * Rw], F16, tag="s_wa")
                        nc.sync.dma_start(
                            out=whe[:],
                            in_=d_wp["he"].ap()[b * 128:(b + 1) * 128, :]
                            .rearrange("p (i r) -> p i r", i=4)[:, :, cs])
                        for i in range(4):
                            for e in range(ET):
                                nc.tensor.matmul(ps_he[e][:],
                                                 xe_t[:, i * E + e * 128: i * E + (e + 1) * 128],
                                                 whe[:, i * Rw:(i + 1) * Rw],
                                                 start=(b == 0 and i == 0), stop=(b == MB - 1 and i == 3))
                    for e in range(ET):
                        nc.scalar.copy(the_tiles[e][:, cs], ps_he[e][:])

                # ---- branch he T-side projection; tail pipelined per tile
                w_khe = load_w("k_he")

                def ffn_residents():
                    """Issued only after pass2's stream DMAs: the 16KB slab
                    loads occupy the SP queue ~6us each and pass2 is already
                    near the HBM bandwidth limit."""
                    wf1r = xs.tile([128, FT * ET * 128], F16, tag="wf1r")
                    nc.sync.dma_start(
                        out=wf1r[:].rearrange("p (f c) -> p f c", f=FT),
                        in_=d_wf1.ap().rearrange("(f p) c -> p f c", f=FT))
                    wf2r = xs.tile([128, FT * E], F16, tag="wf2r")
                    nc.sync.dma_start(
                        out=wf2r[:].rearrange("p (f c) -> p f c", f=FT),
                        in_=d_wf2.ap()[0:F, :].rearrange("(f p) c -> p f c", f=FT))
                    b1 = mp.tile([128, FT], F32, tag="b1")
                    nc.sync.dma_start(out=b1[:], in_=d_b1t.ap())
                    ones_t = mp.tile([1, 128], F16, tag="ones")
                    nc.sync.dma_start(out=ones_t[:], in_=d_ones.ap())
                    wtb = mp.tile([128, E], F16, tag="wf2b")
                    nc.sync.dma_start(out=wtb[:], in_=d_wf2.ap()[F:F + 128, :])
                    return wf1r, wf2r, b1, ones_t, wtb

                yT = [xa.tile([128, R], F16, tag=f"yT{e}", name=f"yT{e}") for e in range(ET)]
                y_tiles = {}

                def rms(t):
                    xt = x_tiles[t]
                    scr = sp2.tile([128, E], F32, tag="tmp")
                    nc.scalar.activation(scr[:], xt[:], AF.Square)
                    ms = sp2.tile([128, 1], F32, tag="ms")
                    nc.vector.reduce_sum(out=ms[:], in_=scr[:], axis=AX.X)
                    sd = sp2.tile([128, 1], F32, tag="sd")
                    nc.scalar.activation(sd[:], ms[:], AF.Sqrt, scale=1.0 / E, bias=eps_t[:])
                    inv2 = sp2.tile([128, 1], F32, tag="inv")
                    nc.vector.reciprocal(inv2[:], sd[:])
                    yt = sp.tile([128, E], F16, tag="y")
                    nc.vector.tensor_scalar_mul(yt[:], xt[:], inv2[:])
                    y_tiles[t] = yt

                def transpose_tile(t):
                    yt = y_tiles[t]
                    for e in range(ET):
                        pst = pp.tile([128, 128], F16, tag=f"bank{2 + (e % 2)}")
                        nc.tensor.transpose(pst[:], yt[:, e * 128:(e + 1) * 128], identb[:])
                        nc.scalar.copy(yT[e][:, t * 128:(t + 1) * 128], pst[:])

                def ffn_tile(t):
                    wf1r, wf2r, b1, ones_t, wtb = ffn_res
                    pso = pp.tile([128, E], F32, tag=f"bank{4 + t}", name=f"pso{t}")
                    for f in range(FT):
                        psz = pp.tile([128, 128], F32, tag=f"bank{f % 2}")
                        for k in range(ET):
                            nc.tensor.matmul(psz[:],
                                             wf1r[:, f * E + k * 128: f * E + (k + 1) * 128],
                                             yT[k][:, t * 128:(t + 1) * 128],
                                             start=(k == 0), stop=(k == ET - 1))
                        zt = fs.tile([128, 128], F16, tag="zT")
                        nc.scalar.activation(zt[:], psz[:],
                                             AF.Identity if sim_safe else AF.Gelu,
                                             bias=b1[:, f:f + 1])
                        nc.tensor.matmul(pso[:], zt[:], wf2r[:, f * E:(f + 1) * E],
                                         start=(f == 0), stop=False)
                    nc.tensor.matmul(pso[:], ones_t[0:1, :], wtb[0:1, :], start=False, stop=True)
                    ot = sp.tile([128, E], F32, tag="ot")
                    nc.scalar.copy(ot[:], pso[:])
                    nc.sync.dma_start(out=d_out.ap()[t * 128:(t + 1) * 128, :], in_=ot[:])

                pass2_part(0, R)
                ffn_res = ffn_residents()
                w_qhe = load_w("q_he")
                w_vhe = load_w("v_he")
                proj_group([("k_he", src_T("he"), w_khe),
                            ("q_he", src_xnt, w_qhe),
                            ("v_he", src_xnt, w_vhe)], banks=[0, 1, 2])
                for t in range(NT):
                    sdpa("he", t, first=False)
                    rms(t)
                    transpose_tile(t)
                    ffn_tile(t)

            if loopn > 1:
                with tc.For_i(0, loopn, 1) as _i:
                    body(_i)
            else:
                body()

    nc.compile()
    return nc


def _prep_inputs(inputs, scale=1):
    """Host-side folding + sharding. Returns per-core in_maps."""
    dm = _dims(scale)
    N, M, E, R, F, FT, ET = dm["N"], dm["M"], dm["E"], dm["R"], dm["F"], dm["FT"], dm["ET"]
    x_node = np.asarray(inputs["x_node"], np.float32)
    x_edge = np.asarray(inputs["x_edge"], np.float32)
    adj = np.asarray(inputs["adj"], np.float32)
    g_n = np.asarray(inputs["g_n"], np.float32)
    g_e = np.asarray(inputs["g_e"], np.float32)
    g2 = np.asarray(inputs["g2"], np.float32)

    inv_n = (1.0 / np.sqrt((x_node.astype(np.float64) ** 2).mean(axis=1) + 1e-6)).astype(np.float32)
    inv_e = (1.0 / np.sqrt((x_edge.astype(np.float64) ** 2).mean(axis=1) + 1e-6)).astype(np.float32)
    xn_s = x_node * inv_n[:, None]
    xe_s = x_edge * inv_e[:, None]

    perm = np.array([(j % H) * D + j // H for j in range(E)])  # newcol j=(d,g) <- oldcol g*D+d

    def fold_q(w, g):
        return (g[:, None] * np.asarray(w, np.float32)) / np.sqrt(D)

    def fold_k(w, g):
        return g[:, None] * np.asarray(w, np.float32)

    def fold_v(w, g):
        return (g[:, None] * np.asarray(w, np.float32))[:, perm]

    wmap = {
        "q_hh": fold_q(inputs["Wq_hh"], g_n),
        "q_ee": fold_q(inputs["Wq_ee"], g_e),
        "k_ee": fold_k(inputs["Wk_ee"], g_e),
        "q_eh": fold_q(inputs["Wq_eh"], g_e),
        "k_he": fold_k(inputs["Wk_he"], g_e),
        "k_hh": fold_k(inputs["Wk_hh"], g_n),
        "v_hh": fold_v(inputs["Wv_hh"], g_n),
        "v_ee": fold_v(inputs["Wv_ee"], g_n),
        "k_eh": fold_k(inputs["Wk_eh"], g_n),
        "v_eh": fold_v(inputs["Wv_eh"], g_n),
        "q_he": fold_q(inputs["Wq_he"], g_n),
        "v_he": fold_v(inputs["Wv_he"], g_n),
    }
    wf1 = g2[:, None] * np.asarray(inputs["Wf1"], np.float32)
    bf1 = np.asarray(inputs["bf1"], np.float32)
    b1t = np.ascontiguousarray(bf1.reshape(FT, 128).T)
    wf2e = np.zeros((F + 128, E), np.float32)
    wf2e[:F] = np.asarray(inputs["Wf2"], np.float32)
    wf2e[F] = np.asarray(inputs["bf2"], np.float32)

    def batch4(a, width):
        """[T*128, width] -> [T/4*128, 4*width]: 4 consecutive k-tiles side by side."""
        T = a.shape[0] // 128
        return np.ascontiguousarray(
            a.reshape(T // 4, 4, 128, width).transpose(0, 2, 1, 3).reshape(T // 4 * 128, 4 * width))

    def warr(w):
        """[E, E] -> [128, ET*E]: k-tiles along columns."""
        return np.ascontiguousarray(
            np.asarray(w).reshape(ET, 128, E).transpose(1, 0, 2).reshape(128, ET * E))

    shared = {
        "xe4": batch4(xe_s.astype(np.float16), E),
        "xn4": batch4(xn_s.astype(np.float16), E),
        "b1t": b1t,
        "wf1": np.ascontiguousarray(
            wf1.reshape(ET, 128, FT, 128).transpose(2, 1, 0, 3).reshape(FT * 128, ET * 128)).astype(np.float16),
        "wf2": wf2e.astype(np.float16),
        "ident": np.eye(128, dtype=np.float16),
        "onesrow": np.ones((1, 128), np.float16),
    }
    for k, v in wmap.items():
        shared[f"w_{k}"] = warr(v).astype(np.float16)

    wp1 = {b: np.asarray(inputs[f"Wp1_{b}"], np.float32) for b in ("ee", "eh", "he")}
    in_maps = []
    for c in range(NCORES):
        rows = slice(c * R, (c + 1) * R)
        m = dict(shared)
        m["adj4"] = batch4(np.ascontiguousarray(adj[rows].T).astype(np.float16), R)
        xnt = np.ascontiguousarray(xn_s[rows].T)  # [E, R]
        m["xnt4"] = np.ascontiguousarray(
            xnt.reshape(ET, 128, R).transpose(1, 0, 2).reshape(128, ET * R)).astype(np.float16)
        for b in ("ee", "eh", "he"):
            m[f"wp4_{b}"] = batch4(np.ascontiguousarray(wp1[b][rows].T).astype(np.float16), R)
        in_maps.append(m)
    return in_maps


def kernel(**inputs) -> np.ndarray:
    from concourse.bass_utils import run_bass_kernel_spmd

    if "nc" not in _CACHE:
        _CACHE["nc"] = _build()
    nc = _CACHE["nc"]
    in_maps = _prep_inputs(inputs)
    res = run_bass_kernel_spmd(nc, in_maps, list(range(NCORES)))
    out = np.concatenate([res.results[c]["out"] for c in range(NCORES)], axis=0)
    return np.ascontiguousarray(out, dtype=np.float32)
